# revision 22
# baseline (speedup 1.0000x reference)
"""Block cross-attention Trainium2 kernel (Bass/Tile), 8-core SPMD.

Reference computation (see problem statement):
  pooled = mean-pool x over blocks of 16 tokens        [B, nb, D]
  q = pooled @ Wq (16 heads), k/v = enc @ Wk/Wv (4 kv heads, GQA)
  p = softmax(q k^T * scale + mask)                    per kv-head group
  o = p @ v ; out = repeat(o @ Wo, 16 tokens/block)    [B, L, D]

Sharding: 8 cores = (batch b in {0,1}) x (block-range r in {0..3}).
Each core owns 128 query blocks (2048 tokens) of one batch and computes
ALL heads for them, so the output projection finishes on-device with no
cross-core reduction.  The kv projection (full S for all 4 kv heads) is
recomputed per core; softmax work is perfectly sharded.

Matmul inputs are float32r (TF32-like fast fp32): 1 cycle/row streaming
vs 4 for fp32.  The BIR verifier requires producers to emit f32r, so
SBUF tiles feeding matmuls are allocated as f32r.

Device pipeline per core:
  pool:   x-slice [2048, D] -> pooled-sum [128 blocks, D]  (DVE tree add;
          the /16 is folded into the exp scale)
  q:      pooledT (PE transpose) @ Wq -> q [128, 1024] -> per-head PE
          transposes -> qT_g [64, 4*128] per kv-group
  kv:     per 512-row strip of enc: PE transpose -> encT, kvT_g[128, NVP]
          resident = (Wk_g|Wv_g)^T @ encT
  attn:   per kv-group, per 128-row chunk: sT = kT^T @ qT_g, p =
          exp(scale*sT + maskbias) on ACT, v_aug = (v|1) via PE transpose,
          oT_g += v_aug^T @ p in PSUM (ones column accumulates the softmax
          denominator for free)
  norm:   oTn_g = oT_g[0:64] * bcast(1/oT_g[64])  (PE broadcast)
  outp:   out_blocks [128, D] = sum_h oTn^T @ Wo_h ; broadcast each block
          row to its 16 tokens on the DMA out.
"""

import numpy as np

import concourse.bass as bass
import concourse.tile as tile
from concourse import bacc, mybir
from concourse.bass_utils import run_bass_kernel_spmd
from concourse.masks import make_identity

F32 = mybir.dt.float32
F32R = mybir.dt.float32r
I32 = mybir.dt.int32
EXP = mybir.ActivationFunctionType.Exp
ALU = mybir.AluOpType

B, L, S, D = 2, 8192, 4096, 1024
H, HKV, HD, BS = 16, 4, 64, 16
HPG = H // HKV          # 4 q-heads per kv-group
NB = L // BS            # 512 blocks per batch
NBS = NB // 4           # 128 blocks per core
LS = NBS * BS           # 2048 tokens per core
SCALE_EFF = float(1.0 / (np.sqrt(HD) * BS))  # attn scale with /16 pooling folded in
NEG = 30000.0           # (mask-1)*NEG as exp bias kills masked columns

USE_F32R = True
DT = F32R if USE_F32R else F32
COMPACT = True          # gather only mask-valid kv rows (device-side)
NVP = 2304 if COMPACT else S  # kv positions processed (2048 expected + 8 sigma pad)
NCH = NVP // 128        # 128-row score chunks
STRIPS = []             # (start_chunk, n_chunks) kv strips of <=512 cols
_c = 0
while _c < NCH:
    n = min(4, NCH - _c)
    STRIPS.append((_c, n))
    _c += n
TRASH = NVP + 32        # scatter slot for masked-out positions


def emit(nc, tc, ctx):
    x = nc.dram_tensor("x", [LS, D], F32, kind="ExternalInput").ap()
    enc = nc.dram_tensor("enc", [S, D], F32, kind="ExternalInput").ap()
    mask = nc.dram_tensor("mask", [S], I32, kind="ExternalInput").ap()
    wq = nc.dram_tensor("wq", [D, H * HD], F32, kind="ExternalInput").ap()
    wk = nc.dram_tensor("wk", [D, HKV * HD], F32, kind="ExternalInput").ap()
    wv = nc.dram_tensor("wv", [D, HKV * HD], F32, kind="ExternalInput").ap()
    wo = nc.dram_tensor("wo", [H * HD, D], F32, kind="ExternalInput").ap()
    out = nc.dram_tensor("out", [LS, D], F32, kind="ExternalOutput").ap()

    ctx.enter_context(nc.allow_low_precision(reason="f32r matmul inputs"))
    res = ctx.enter_context(tc.tile_pool(name="res", bufs=1))
    big = ctx.enter_context(tc.tile_pool(name="big", bufs=2))
    sm = ctx.enter_context(tc.tile_pool(name="sm", bufs=2))
    ps = ctx.enter_context(tc.tile_pool(name="ps", bufs=2, space="PSUM"))

    # f32r constants must be produced by a rounding op (DVE copy from f32)
    identf = res.tile([128, 128], F32, tag="identf")
    make_identity(nc, identf[:])
    ident = res.tile([128, 128], DT, tag="ident")
    nc.vector.tensor_copy(ident[:], identf[:])
    # identity block on partitions 64-127 (for transposing base-64 slices)
    ident2f = res.tile([128, 64], F32, tag="ident2f")
    nc.gpsimd.memset(ident2f[:], 0.0)
    nc.gpsimd.affine_select(
        out=ident2f[:],
        in_=ident2f[:],
        compare_op=ALU.not_equal,
        fill=1.0,
        base=-64,
        pattern=[[-1, 64]],
        channel_multiplier=1,
    )
    ident2 = res.tile([128, 64], DT, tag="ident2")
    nc.vector.tensor_copy(ident2[:], ident2f[:])
    onesf = res.tile([128, 1], F32, tag="onesf")
    nc.gpsimd.memset(onesf[:], 1.0)
    vones = res.tile([128, 1], DT, tag="vones")
    nc.vector.tensor_copy(vones[:], onesf[:])

    # ---- mask -> compact kv index list (device-side stream compaction) ----
    biasb = res.tile([128, NCH], F32, tag="biasb")
    if COMPACT:
        dram = ctx.enter_context(tc.tile_pool(name="dram", bufs=1, space="DRAM"))
        idxd = dram.tile([NVP + 128, 1], I32, tag="idxd")
        # mask in two layouts (s = 32p + i)
        mt_i = res.tile([32, 128], I32, tag="mt_i")
        nc.sync.dma_start(mt_i[:], mask.rearrange("(p i) -> i p", i=32))
        mt_f = res.tile([32, 128], F32, tag="mt_f")
        nc.vector.tensor_copy(mt_f[:], mt_i[:])
        m_i = res.tile([128, 32], I32, tag="m_i")
        nc.sync.dma_start(m_i[:], mask.rearrange("(p i) -> p i", i=32))
        m_f = res.tile([128, 32], F32, tag="m_f")
        nc.vector.tensor_copy(m_f[:], m_i[:])
        # upper-triangular (incl) [32, 32]: U[i, j] = 1 if i <= j
        u32f = res.tile([32, 32], F32, tag="u32f")
        nc.gpsimd.memset(u32f[:], 0.0)
        nc.gpsimd.affine_select(
            out=u32f[:], in_=u32f[:], compare_op=ALU.is_gt, fill=1.0,
            base=0, pattern=[[-1, 32]], channel_multiplier=1,
        )
        u32r = res.tile([32, 32], F32, tag="u32r")
        nc.vector.tensor_copy(u32r[:], u32f[:])
        # strict lower-triangular [128, 128]: L[q, p] = 1 if q < p
        lf = res.tile([128, 128], F32, tag="lf")
        nc.gpsimd.memset(lf[:], 0.0)
        nc.gpsimd.affine_select(
            out=lf[:], in_=lf[:], compare_op=ALU.is_ge, fill=1.0,
            base=0, pattern=[[-1, 128]], channel_multiplier=1,
        )
        lr = res.tile([128, 128], F32, tag="lr")
        nc.vector.tensor_copy(lr[:], lf[:])
        # local inclusive cumsum per 32-chunk, then chunk-offset prefix
        loc = ps.tile([128, 32], F32, tag="tp")
        nc.tensor.matmul(loc[:], lhsT=mt_f[:], rhs=u32r[:], start=True, stop=True)
        totr = res.tile([128, 1], F32, tag="totr")
        nc.vector.tensor_copy(totr[:], loc[:, 31:32])
        offs = ps.tile([128, 1], F32, tag="tp")
        nc.tensor.matmul(offs[:], lhsT=lr[:], rhs=totr[:], start=True, stop=True)
        offs_sb = res.tile([128, 1], F32, tag="offs_sb")
        nc.vector.tensor_copy(offs_sb[:], offs[:])
        # slot = m ? (offs + loc - m) : TRASH
        e1 = res.tile([128, 32], F32, tag="e1")
        nc.vector.tensor_tensor(out=e1[:], in0=loc[:], in1=m_f[:], op=ALU.subtract)
        nc.vector.tensor_scalar(
            out=e1[:], in0=e1[:], scalar1=offs_sb[:, 0:1], scalar2=float(-TRASH),
            op0=ALU.add, op1=ALU.add,
        )
        nc.vector.tensor_tensor(out=e1[:], in0=e1[:], in1=m_f[:], op=ALU.mult)
        nc.vector.tensor_scalar_add(out=e1[:], in0=e1[:], scalar1=float(TRASH))
        slot_i = res.tile([128, 32], I32, tag="slot_i")
        nc.vector.tensor_copy(slot_i[:], e1[:])
        # iota values = s position
        iv = res.tile([128, 32], I32, tag="iv")
        nc.gpsimd.iota(iv[:], pattern=[[1, 32]], base=0, channel_multiplier=32)
        # zero the table, scatter s into compact slots
        zt = res.tile([128, 19], I32, tag="zt")
        nc.gpsimd.memset(zt[:], 0)
        idv = idxd[:].rearrange("(c p) w -> p (c w)", p=128)
        nc.sync.dma_start(idv[:, 0:19], zt[:])
        for i in range(32):
            nc.gpsimd.indirect_dma_start(
                out=idxd[:],
                out_offset=bass.IndirectOffsetOnAxis(ap=slot_i[:, i : i + 1], axis=0),
                in_=iv[:, i : i + 1],
                in_offset=None,
            )
        # reload compact idx [128, NCH]; recover pad mask: idx>0 or slot 0
        idx_sb = res.tile([128, NCH], I32, tag="idx_sb")
        nc.sync.dma_start(idx_sb[:], idv[:, 0:NCH])
        idxf = res.tile([128, NCH], F32, tag="idxf")
        nc.vector.tensor_copy(idxf[:], idx_sb[:])
        mcf = res.tile([128, NCH], F32, tag="mcf")
        nc.vector.tensor_scalar(
            out=mcf[:], in0=idxf[:], scalar1=0.0, scalar2=None, op0=ALU.is_gt
        )
        nc.vector.memset(mcf[0:1, 0:1], 1.0)
        nc.vector.tensor_scalar(
            out=biasb[:], in0=mcf[:], scalar1=1.0, scalar2=NEG,
            op0=ALU.subtract, op1=ALU.mult,
        )
    else:
        mi = res.tile([128, NCH], I32, tag="mi")
        nc.sync.dma_start(mi[:], mask.rearrange("(c p) -> p c", p=128))
        mf = res.tile([128, NCH], F32, tag="mf")
        nc.vector.tensor_copy(mf[:], mi[:])
        nc.vector.tensor_scalar(
            out=biasb[:], in0=mf[:], scalar1=1.0, scalar2=NEG,
            op0=ALU.subtract, op1=ALU.mult,
        )

    # ---- pooling: x [2048, D] -> pooled sum [128 blocks, D] ----
    pooled = res.tile([128, D], DT, tag="pooled")
    acc = res.tile([128, D], F32, tag="acc")
    xv = x.rearrange("(p j) d -> p j d", j=BS)  # [128, 16, 1024]
    for r in range(8):
        xt = big.tile([128, 2 * D], F32, tag="xt")
        nc.sync.dma_start(
            xt[:].rearrange("p (j d) -> p j d", j=2), xv[:, 2 * r : 2 * r + 2, :]
        )
        h2 = big.tile([128, D], F32, tag="h2")
        nc.vector.tensor_add(h2[:], xt[:, 0:D], xt[:, D : 2 * D])
        if r == 0:
            nc.vector.tensor_copy(acc[:], h2[:])
        elif r < 7:
            nc.vector.tensor_add(acc[:], acc[:], h2[:])
        else:
            nc.vector.tensor_add(pooled[:], acc[:], h2[:])

    # ---- q projection: q [128 blocks, H*HD] ----
    pooledT = res.tile([128, D], DT, tag="pooledT")  # 8 chunks of [128d, 128blk]
    for dc in range(8):
        tp = ps.tile([128, 128], DT, tag="tp")
        nc.tensor.transpose(tp[:], pooled[:, 128 * dc : 128 * (dc + 1)], ident[:])
        nc.vector.tensor_copy(pooledT[:, 128 * dc : 128 * (dc + 1)], tp[:])

    qps = [ps.tile([128, 512], F32, name=f"qps{i}", tag=f"mm{i}") for i in range(2)]
    for dc in range(8):
        wqt = big.tile([128, H * HD], DT, tag="wqt")
        nc.sync.dma_start(wqt[:], wq[128 * dc : 128 * (dc + 1), :].bitcast(DT))
        for half in range(2):
            nc.tensor.matmul(
                qps[half][:],
                lhsT=pooledT[:, 128 * dc : 128 * (dc + 1)],
                rhs=wqt[:, 512 * half : 512 * (half + 1)],
                start=(dc == 0),
                stop=(dc == 7),
            )
    qsb = res.tile([128, H * HD], DT, tag="qsb")
    for half in range(2):
        nc.vector.tensor_copy(qsb[:, 512 * half : 512 * (half + 1)], qps[half][:])

    # qT_g [64, HPG*128] per kv-group
    qT = [
        res.tile([64, HPG * 128], DT, name=f"qT{g}", tag=f"qT{g}") for g in range(HKV)
    ]
    for g in range(HKV):
        for h in range(HPG):
            hh = g * HPG + h
            tp = ps.tile([128, 128], DT, tag="tp")
            nc.tensor.transpose(
                tp[0:64, 0:128], qsb[:, 64 * hh : 64 * (hh + 1)], ident[:]
            )
            nc.vector.tensor_copy(qT[g][:, 128 * h : 128 * (h + 1)], tp[0:64, 0:128])

    # ---- kv weights resident: wkv_g chunks [128 D, 64 k | 64 v] ----
    wkvt = []
    for g in range(HKV):
        row = []
        for dc in range(8):
            t = res.tile([128, 128], DT, name=f"wkv{g}_{dc}", tag=f"wkv{g}_{dc}")
            nc.sync.dma_start(
                t[:, 0:64], wk[128 * dc : 128 * (dc + 1), 64 * g : 64 * (g + 1)].bitcast(DT)
            )
            nc.sync.dma_start(
                t[:, 64:128], wv[128 * dc : 128 * (dc + 1), 64 * g : 64 * (g + 1)].bitcast(DT)
            )
            row.append(t)
        wkvt.append(row)

    # ---- kv projection: kvT_g [128, NVP] resident, per 512-col strip ----
    kvT = [
        res.tile([128, NVP], DT, name=f"kvT{g}", tag=f"kvT{g}") for g in range(HKV)
    ]
    for st, (c0, nch) in enumerate(STRIPS):
        sw = 128 * nch
        encT = [
            big.tile([128, 512], DT, name=f"encT{st}_{dc}", tag=f"encT{dc}", bufs=1)
            for dc in range(8)
        ]
        for sc in range(nch):
            et = big.tile([128, D], DT, tag="et")
            if COMPACT:
                nc.gpsimd.indirect_dma_start(
                    out=et[:],
                    out_offset=None,
                    in_=enc[:, :].bitcast(DT),
                    in_offset=bass.IndirectOffsetOnAxis(
                        ap=idx_sb[:, c0 + sc : c0 + sc + 1], axis=0
                    ),
                )
            else:
                r0 = (c0 + sc) * 128
                nc.sync.dma_start(et[:], enc[r0 : r0 + 128, :].bitcast(DT))
            for dc in range(8):
                tp = ps.tile([128, 128], DT, tag="tp")
                nc.tensor.transpose(tp[:], et[:, 128 * dc : 128 * (dc + 1)], ident[:])
                nc.vector.tensor_copy(encT[dc][:, 128 * sc : 128 * (sc + 1)], tp[:])
        for g in range(HKV):
            kvp = ps.tile([128, 512], F32, name=f"kvp{st}_{g}", tag="mm0")
            for dc in range(8):
                nc.tensor.matmul(
                    kvp[:, 0:sw],
                    lhsT=wkvt[g][dc][:],
                    rhs=encT[dc][:, 0:sw],
                    start=(dc == 0),
                    stop=(dc == 7),
                )
            nc.vector.tensor_copy(kvT[g][:, 128 * c0 : 128 * (c0 + nch)], kvp[:, 0:sw])

    # ---- attention: sT -> exp -> oT accumulated in PSUM ----
    oTn = [res.tile([64, 512], DT, name=f"oTn{g}", tag=f"oTn{g}") for g in range(HKV)]
    ones64 = res.tile([1, 64], DT, tag="ones64")
    nc.vector.tensor_copy(ones64[:], onesf[0:1, 0:1].to_broadcast([1, 64]))
    for g in range(HKV):
        otp = ps.tile([65, 512], F32, name=f"otp{g}", tag="otp")
        for c in range(NCH):
            sps = ps.tile([128, 512], F32, name=f"sps{g}_{c}", tag="mm1")
            nc.tensor.matmul(
                sps[:],
                lhsT=kvT[g][0:64, 128 * c : 128 * (c + 1)],
                rhs=qT[g][:],
                start=True,
                stop=True,
            )
            pt = sm.tile([128, 512], DT, tag="pt")
            nc.scalar.activation(
                pt[:], sps[:], EXP, bias=biasb[:, c : c + 1], scale=SCALE_EFF
            )
            vtp = ps.tile([128, 128], DT, tag="tp")
            nc.tensor.transpose(
                vtp[0:128, 0:64],
                kvT[g][64:128, 128 * c : 128 * (c + 1)],
                ident2[64:128, 0:64],
            )
            va = sm.tile([128, 65], DT, tag="va")
            nc.vector.tensor_copy(va[:, 0:64], vtp[0:128, 0:64])
            nc.vector.tensor_copy(va[:, 64:65], vones[:])
            nc.tensor.matmul(
                otp[:], lhsT=va[:], rhs=pt[:], start=(c == 0), stop=(c == NCH - 1)
            )
        # normalize: oTn_g = otp[0:64] * bcast(1/otp[64])
        rec = sm.tile([1, 512], DT, tag="rec")
        nc.vector.reciprocal(rec[:], otp[64:65, :])
        bc = ps.tile([64, 512], F32, name=f"bc{g}", tag="mm0")
        nc.tensor.matmul(bc[:], lhsT=ones64[:], rhs=rec[:], start=True, stop=True)
        bcs = sm.tile([64, 512], F32, tag="bcs")
        nc.vector.tensor_copy(bcs[:], bc[:])
        nc.vector.tensor_tensor(
            out=oTn[g][:], in0=otp[0:64, :], in1=bcs[:], op=ALU.mult
        )

    # ---- output projection + broadcast write ----
    ops_ = [ps.tile([128, 512], F32, name=f"ops{i}", tag=f"mm{i}") for i in range(2)]
    for hh in range(H):
        g, h = hh // HPG, hh % HPG
        wot = sm.tile([64, D], DT, tag="wot")
        nc.sync.dma_start(wot[:], wo[64 * hh : 64 * (hh + 1), :].bitcast(DT))
        for half in range(2):
            nc.tensor.matmul(
                ops_[half][:],
                lhsT=oTn[g][:, 128 * h : 128 * (h + 1)],
                rhs=wot[:, 512 * half : 512 * (half + 1)],
                start=(hh == 0),
                stop=(hh == H - 1),
            )
    osb = res.tile([128, D], F32, tag="osb")
    for half in range(2):
        nc.vector.tensor_copy(osb[:, 512 * half : 512 * (half + 1)], ops_[half][:])
    ov = out.rearrange("(p j) d -> p j d", j=BS)
    for j in range(BS):
        nc.sync.dma_start(ov[:, j, :], osb[:])


_CACHE = {}


def _build():
    if "nc" not in _CACHE:
        from contextlib import ExitStack

        nc = bacc.Bacc("TRN2", target_bir_lowering=False, debug=False, num_devices=8)
        with tile.TileContext(nc) as tc, ExitStack() as ctx:
            emit(nc, tc, ctx)
        nc.compile()
        _CACHE["nc"] = nc
    return _CACHE["nc"]


def kernel(x, enc, mask, Wq, Wk, Wv, Wo):
    nc = _build()
    in_maps = []
    for core in range(8):
        b, r = core // 4, core % 4
        in_maps.append(
            {
                "x": np.ascontiguousarray(x[b, r * LS : (r + 1) * LS, :], np.float32),
                "enc": np.ascontiguousarray(enc[b], np.float32),
                "mask": np.ascontiguousarray(mask[b], np.int32),
                "wq": np.asarray(Wq, np.float32),
                "wk": np.asarray(Wk, np.float32),
                "wv": np.asarray(Wv, np.float32),
                "wo": np.asarray(Wo, np.float32),
            }
        )
    res = run_bass_kernel_spmd(nc, in_maps, core_ids=list(range(8)))
    out = np.empty((B, L, D), np.float32)
    for core in range(8):
        b, r = core // 4, core % 4
        out[b, r * LS : (r + 1) * LS, :] = res.results[core]["out"]
    return out
